# revision 18
# baseline (speedup 1.0000x reference)
"""GCN layer kernel for Trainium2, 8 NeuronCores — single launch.

Math (identical to reference):
    deg = bincount(row);  d = 1/sqrt(deg)
    h   = x @ W.T + b
    out = d * segment_sum(d[col] * h[col], row) + d^2 * h

Aggregate-then-transform (linear map commutes with the segment sum):
    y[j]   = d_j * x_j                      (host, bf16)
    U[r]   = sum_{edges (r,c)} y[c] + y[r]  (self term = extra edge slot)
    cc[r]  = sum_{edges (r,c)} d_c + d_r    (host)
    out[r] = d_r * (U[r] @ W.T + cc[r] * b)

Device program (SPMD over 8 cores, destinations sharded):
  * destinations are dealt to (core, position) round-robin in descending
    degree order, which equalizes per-core work per gather call and so
    minimizes the cross-core-max padding the static SPMD schedule needs.
  * edges sorted by (dest superblock of SBD, source chunk, dest); bulk
    gathered with gpsimd.dma_gather (256B bf16 y rows) spread round-robin
    over 4 SWDGE queues (a single queue runs at only ~27 GB/s).  Gathered
    edge i lands at SBUF partition i%128, tile i//128.
  * per 128-edge tile a 0/1 selection matrix S[edge, dest_local] over the
    tile's dest window is PRECOMPUTED ON HOST (bf16) and streamed in via
    regular DMA (~20 MB/core; cheaper than building it on DVE, which costs
    ~290 ns/instruction on HW).  One PE matmul per tile accumulates
    G^T @ S into the 512-dest half's PSUM bank as U^T [feat, dest] — the
    final W matmul reads that as lhsT directly, so there are no transposes.
  * per 128-dest stripe: o2 = U_T_stripe^T @ W^T plus a rank-1 cc x b
    matmul into the same PSUM, then one activation copy scaled by d -> out.
"""

import numpy as np
import sys

sys.path.insert(0, "/opt/trn_rl_repo")

import concourse.bacc as bacc
import concourse.tile as tile
from concourse import mybir
from concourse.bass_utils import run_bass_kernel_spmd

NCORES = 8
P = 128
MAXCHUNK = 32000  # dma_gather idx is int16: source chunks must stay < 32768 rows
SBD = 1024  # dests per superblock (gather-slab granularity)
HW = 512  # dests per PSUM half (one 2KB fp32 bank)
NQUEUES = 4
F32 = mybir.dt.float32
I16 = mybir.dt.int16
BF16 = mybir.dt.bfloat16
FP8 = mybir.dt.float8e4

_cache = {}
LAST = {}  # populated on each kernel() call (for profiling in test.py)


def _build(meta, nrep=1, mode="full"):
    din = meta["din"]
    dout = meta["dout"]
    chunk = meta["chunk"]
    n_y = meta["n_y"]
    npc_pad = meta["npc_pad"]
    nblk = meta["nblk"]  # 128-dest stripes per core
    nsb = meta["nsb"]
    nhalf = meta["nhalf"]
    ttot = meta["ttot"]
    stot = meta["stot"]
    sb_calls = meta["sb_calls"]  # per sb: list of (chunk, tile_off_in_sb, ntiles)
    sb_base = meta["sb_base"]  # per sb: global tile offset
    s_base = meta["s_base"]  # per sb: S column offset
    sw_sb = meta["sw_sb"]  # per sb: S columns
    max_sb_tiles = meta["max_sb_tiles"]
    sb_tiles = meta["sb_tiles"]
    max_sw = meta["max_sw"]
    half_tiles = meta["half_tiles"]  # per half: list of (tile_in_sb, lo, w, soff_in_sb)

    nc = bacc.Bacc(
        "TRN2",
        target_bir_lowering=False,
        debug=False,
        enable_asserts=False,
        num_devices=NCORES,
        num_swdge_queues=NQUEUES,
    )
    y_t = nc.dram_tensor("y_t", [n_y, din], BF16, kind="ExternalInput").ap()
    idx_t = nc.dram_tensor("idx_t", [P, ttot * 8], I16, kind="ExternalInput").ap()
    s_t = nc.dram_tensor("s_t", [P, stot], FP8, kind="ExternalInput").ap()
    wt_t = nc.dram_tensor("wt_t", [din, dout], F32, kind="ExternalInput").ap()
    brow_t = nc.dram_tensor("brow_t", [1, dout], BF16, kind="ExternalInput").ap()
    ccrow_t = nc.dram_tensor("ccrow_t", [1, npc_pad], BF16, kind="ExternalInput").ap()
    dsb_t = nc.dram_tensor("dsb_t", [P, nblk], F32, kind="ExternalInput").ap()
    out_t = nc.dram_tensor("out_t", [npc_pad, dout], F32, kind="ExternalOutput").ap()
    out_v = out_t.rearrange("(t p) f -> p t f", p=P)

    with tile.TileContext(nc) as tc:
        with (
            tc.tile_pool(name="const", bufs=1) as cpool,
            tc.tile_pool(name="slab", bufs=2) as gpool,
            tc.tile_pool(name="sslab", bufs=2) as spool,
            tc.tile_pool(name="idxp", bufs=2) as ipool,
            tc.tile_pool(name="work", bufs=3) as wpool,
            tc.tile_pool(name="out", bufs=2) as opool,
            tc.tile_pool(name="psU", bufs=3, space="PSUM") as ppool,
            tc.tile_pool(name="psO", bufs=2, space="PSUM") as p2pool,
        ):
            wt_sb = cpool.tile([din, dout], dtype=F32)
            nc.sync.dma_start(out=wt_sb[:], in_=wt_t[:, :])
            brow_sb = cpool.tile([1, dout], dtype=BF16)
            nc.sync.dma_start(out=brow_sb[:], in_=brow_t[:, :])
            ccrow_sb = cpool.tile([1, npc_pad], dtype=BF16)
            nc.sync.dma_start(out=ccrow_sb[:], in_=ccrow_t[:, :])
            dsb_sb = cpool.tile([P, nblk], dtype=F32)
            nc.sync.dma_start(out=dsb_sb[:], in_=dsb_t[:, :])
            zs = cpool.tile([P, HW], dtype=FP8)
            nc.gpsimd.memset(zs[:], 0.0)

            def body():
                qrr = [0]
                for sb in range(nsb):
                    tb = sb_base[sb]
                    nt_sb = sb_tiles[sb]
                    idx_sb = ipool.tile([P, max_sb_tiles * 8], dtype=I16, tag="idx")
                    nc.gpsimd.dma_start(
                        out=idx_sb[:, 0 : nt_sb * 8],
                        in_=idx_t[:, tb * 8 : (tb + nt_sb) * 8],
                    )
                    slab = gpool.tile([P, max_sb_tiles, din], dtype=BF16, tag="slab")
                    for (c, toff, nt) in sb_calls[sb] if mode != "nogather" else []:
                        c_lo = c * chunk
                        c_hi = min((c + 1) * chunk, n_y)
                        nc.gpsimd.dma_gather(
                            out_ap=slab[:, toff : toff + nt, :],
                            in_ap=y_t[c_lo:c_hi, :],
                            idxs_ap=idx_sb[:, toff * 8 : (toff + nt) * 8],
                            num_idxs=nt * P,
                            num_idxs_reg=nt * P,
                            elem_size=din,
                            single_packet=False,
                            queue_num=qrr[0] % NQUEUES,
                        )
                        qrr[0] += 1
                    sslab = spool.tile([P, max_sw], dtype=FP8, tag="sslab")
                    nc.scalar.dma_start(
                        out=sslab[:, 0 : sw_sb[sb]],
                        in_=s_t[:, s_base[sb] : s_base[sb] + sw_sb[sb]],
                    )
                    for hpar in range(SBD // HW):
                        h = sb * (SBD // HW) + hpar
                        if h >= nhalf:
                            break
                        if mode == "gather":
                            nq = min(HW // P, nblk - h * (HW // P))
                            ob = opool.tile([P, HW // P, dout], dtype=F32, tag="ob")
                            for q in range(nq):
                                nc.scalar.activation(
                                    ob[:, q, :], wt_sb[:],
                                    mybir.ActivationFunctionType.Copy,
                                )
                            nc.sync.dma_start(
                                out=out_v[:, h * (HW // P) : h * (HW // P) + nq, :],
                                in_=ob[:, 0:nq, :],
                            )
                            continue
                        tiles = half_tiles[h]
                        cw = min(HW, npc_pad - h * HW)
                        U = ppool.tile([P, HW], dtype=F32, space="PSUM", tag="U")
                        ntb = len(tiles)
                        nc.tensor.matmul(
                            out=U[:, 0:cw],
                            lhsT=slab[:, 0, :],
                            rhs=zs[:, 0:cw],
                            start=True,
                            stop=False,
                        )
                        for ti, (tloc, lo, w, soff) in enumerate(tiles):
                            nc.tensor.matmul(
                                out=U[:, lo : lo + w],
                                lhsT=slab[:, tloc, :],
                                rhs=sslab[:, soff : soff + w],
                                start=False,
                                stop=(ti == ntb - 1),
                            )
                        ut = wpool.tile([P, HW], dtype=F32, tag="ut")
                        nc.scalar.activation(
                            ut[:], U[:], mybir.ActivationFunctionType.Copy
                        )
                        nq = min(HW // P, nblk - h * (HW // P))
                        ob = opool.tile([P, HW // P, dout], dtype=F32, tag="ob")
                        for q in range(nq):
                            g = h * (HW // P) + q
                            o2 = p2pool.tile([P, dout], dtype=F32, space="PSUM", tag="o2")
                            nc.tensor.matmul(
                                out=o2[:],
                                lhsT=ut[:, q * P : (q + 1) * P],
                                rhs=wt_sb[:],
                                start=True,
                                stop=False,
                            )
                            nc.tensor.matmul(
                                out=o2[:],
                                lhsT=ccrow_sb[0:1, g * P : (g + 1) * P],
                                rhs=brow_sb[0:1, :],
                                start=False,
                                stop=True,
                            )
                            nc.scalar.activation(
                                ob[:, q, :],
                                o2[:],
                                mybir.ActivationFunctionType.Copy,
                                scale=dsb_sb[:, g : g + 1],
                            )
                        nc.sync.dma_start(
                            out=out_v[:, h * (HW // P) : h * (HW // P) + nq, :],
                            in_=ob[:, 0:nq, :],
                        )

            if nrep > 1:
                with tc.For_i(0, nrep, 1):
                    body()
            else:
                body()
    nc.compile()
    return nc


def _prep(x, edge_index, W, b):
    import ml_dtypes

    bf16 = np.dtype(ml_dtypes.bfloat16)
    N, din = x.shape
    dout = W.shape[0]
    npc = N // NCORES
    assert npc * NCORES == N
    nblk = (npc + P - 1) // P
    npc_pad = nblk * P
    nhalf = (npc_pad + HW - 1) // HW
    hpb = SBD // HW
    nsb = (nhalf + hpb - 1) // hpb
    nchunk = (N + MAXCHUNK - 1) // MAXCHUNK
    chunk = (N + nchunk - 1) // nchunk
    n_y = N

    row = np.asarray(edge_index[0], dtype=np.int64)
    col = np.asarray(edge_index[1], dtype=np.int64)
    deg = np.bincount(row, minlength=N)
    d = 1.0 / np.sqrt(deg.astype(np.float64))
    y = (x.astype(np.float64) * d[:, None]).astype(np.float32).astype(bf16)
    cc = d + np.bincount(row, weights=d[col], minlength=N)

    # destination -> (core, position): deal in descending-degree order so all
    # cores see near-identical per-call work (minimizes cross-core-max pad).
    perm = np.argsort(-deg, kind="stable")  # perm[rank] = node
    rank = np.empty(N, dtype=np.int64)
    rank[perm] = np.arange(N)

    # ---- slots = edges + self edges, dest-sharded -------------------------
    rows_a = np.concatenate([row, np.arange(N, dtype=np.int64)])
    cols_a = np.concatenate([col, np.arange(N, dtype=np.int64)])
    r = rank[rows_a]
    core = r % NCORES
    rl = r // NCORES  # dest position within core, 0..npc-1
    sb = rl // SBD
    ch = cols_a // chunk
    sbg = core * nsb + sb
    order = np.lexsort((rl, ch, sbg))
    core_s, rl_s, ch_s, cols_s = core[order], rl[order], ch[order], cols_a[order]
    sb_s = rl_s // SBD
    gid = (core_s * nsb + sb_s) * nchunk + ch_s
    ngrp = NCORES * nsb * nchunk
    gcnt = np.bincount(gid, minlength=ngrp).reshape(NCORES, nsb * nchunk)
    tcnt = (gcnt.max(axis=0) + P - 1) // P  # [nsb*nchunk]
    tile_start = np.zeros(nsb * nchunk + 1, dtype=np.int64)
    np.cumsum(tcnt, out=tile_start[1:])
    ttot = int(tile_start[-1])
    sb_base = [int(tile_start[s * nchunk]) for s in range(nsb)]
    sb_tiles = [
        int(tile_start[(s + 1) * nchunk] - tile_start[s * nchunk]) for s in range(nsb)
    ]
    max_sb_tiles = max(sb_tiles)

    sb_calls = []
    for s in range(nsb):
        calls = []
        for c in range(nchunk):
            nt = int(tcnt[s * nchunk + c])
            if nt:
                calls.append((c, int(tile_start[s * nchunk + c]) - sb_base[s], nt))
        sb_calls.append(calls)

    # ---- per-core slot data ----------------------------------------------
    grp_start = np.zeros(ngrp + 1, dtype=np.int64)
    np.cumsum(np.bincount(gid, minlength=ngrp), out=grp_start[1:])
    rank_in_g = np.arange(len(gid), dtype=np.int64) - grp_start[gid]
    gnc = gid % (nsb * nchunk)
    slot = tile_start[gnc] * P + rank_in_g
    tno = slot // P
    pno = slot - tno * P
    lidx = (cols_s - ch_s * chunk).astype(np.int16)
    dl10 = rl_s - sb_s * SBD  # 0..SBD-1 within superblock

    idx_all = np.zeros((NCORES, P, ttot * 8), dtype=np.int16)
    dl_all = np.full((NCORES, ttot, P), -1.0, dtype=np.float32)
    nkey = ttot * hpb
    wmin = np.full(nkey, SBD, dtype=np.int64)
    wmax = np.full(nkey, -1, dtype=np.int64)
    for m in range(NCORES):
        sel = core_s == m
        flat = np.zeros((ttot, P), dtype=np.int16)
        flat[tno[sel], pno[sel]] = lidx[sel]
        wrapped = flat.reshape(ttot, 8, 16).transpose(2, 0, 1).reshape(16, ttot * 8)
        idx_all[m] = np.tile(wrapped, (8, 1))
        dl_all[m][tno[sel], pno[sel]] = dl10[sel].astype(np.float32)
        key = tno[sel] * hpb + dl10[sel] // HW
        np.minimum.at(wmin, key, dl10[sel])
        np.maximum.at(wmax, key, dl10[sel])

    # ---- per-half tile schedule + packed host-built S ---------------------
    half_tiles = [[] for _ in range(nhalf)]
    s_base = []
    sw_sb = []
    s_entries = []  # (sb, tile, hpar, lo_abs, w, scol)
    scol = 0
    for s in range(nsb):
        s_base.append(scol)
        for hp in range(hpb):
            h = s * hpb + hp
            if h >= nhalf:
                break
            for c in range(nchunk):
                t0, t1 = int(tile_start[s * nchunk + c]), int(tile_start[s * nchunk + c + 1])
                for t in range(t0, t1):
                    k = t * hpb + hp
                    if wmax[k] < 0:
                        continue
                    lo = int(wmin[k]) - hp * HW
                    w = int(wmax[k]) - hp * HW + 1 - lo
                    half_tiles[h].append((t - sb_base[s], lo, w, scol - s_base[s]))
                    s_entries.append((t, hp * HW + lo, w))
                    scol += w
            assert half_tiles[h], f"half {h} has no tiles"
        sw_sb.append(scol - s_base[s])
    stot = scol
    max_sw = max(sw_sb)

    f8 = np.dtype(ml_dtypes.float8_e4m3)
    s_all = np.zeros((NCORES, P, stot), dtype=f8)
    sf = np.empty((P, 512), dtype=np.float32)
    for m in range(NCORES):
        dlm = dl_all[m]
        buf = np.zeros((P, stot), dtype=np.float32)
        for (t, lo_abs, w), sc in zip(s_entries, _scols(s_entries)):
            np.equal(
                dlm[t][:, None],
                np.arange(lo_abs, lo_abs + w, dtype=np.float32)[None, :],
                out=sf[:, 0:w],
            )
            buf[:, sc : sc + w] = sf[:, 0:w]
        s_all[m] = buf.astype(f8)

    # ---- small per-core arrays (permuted node order) ----------------------
    ccrow = np.zeros((NCORES, 1, npc_pad), dtype=bf16)
    dsb = np.ones((NCORES, P, nblk), dtype=np.float32)
    pos = np.arange(npc, dtype=np.int64)
    for m in range(NCORES):
        nodes = perm[pos * NCORES + m]
        ccrow[m, 0, :npc] = cc[nodes].astype(np.float32)
        dm = np.ones(npc_pad, dtype=np.float64)
        dm[:npc] = d[nodes]
        dsb[m] = dm.reshape(nblk, P).T.astype(np.float32)

    meta = dict(
        N=N, din=din, dout=dout, npc=npc, nblk=nblk, npc_pad=npc_pad,
        nchunk=nchunk, chunk=chunk, n_y=n_y, nsb=nsb, nhalf=nhalf, ttot=ttot,
        stot=stot, sb_calls=sb_calls, sb_base=sb_base, s_base=s_base,
        sw_sb=sw_sb, max_sb_tiles=max_sb_tiles, sb_tiles=sb_tiles, max_sw=max_sw,
        half_tiles=half_tiles,
    )
    data = dict(y=y, idx_all=idx_all, s_all=s_all, ccrow=ccrow, dsb=dsb, perm=perm)
    return meta, data


def _scols(s_entries):
    sc = 0
    for (_, _, w) in s_entries:
        yield sc
        sc += w


def _meta_key(meta):
    return (
        meta["N"], meta["din"], meta["dout"], meta["ttot"], meta["stot"],
        meta["max_sb_tiles"], meta["max_sw"],
        tuple(tuple(t) for tl in meta["half_tiles"] for t in tl),
        tuple(tuple(c) for cl in meta["sb_calls"] for c in cl),
    )


def kernel(x, edge_index, W, b):
    x = np.asarray(x, dtype=np.float32)
    W = np.asarray(W, dtype=np.float32)
    b = np.asarray(b, dtype=np.float32)
    edge_index = np.asarray(edge_index)
    meta, data = _prep(x, edge_index, W, b)
    N, din, dout = meta["N"], meta["din"], meta["dout"]

    key = ("l", _meta_key(meta))
    if key not in _cache:
        _cache[key] = _build(meta)
    nc = _cache[key]

    wt = np.ascontiguousarray(W.T)
    import ml_dtypes
    brow = b[None, :].astype(np.dtype(ml_dtypes.bfloat16))
    in_maps = [
        {
            "y_t": data["y"],
            "idx_t": data["idx_all"][m],
            "s_t": data["s_all"][m],
            "wt_t": wt,
            "brow_t": brow,
            "ccrow_t": data["ccrow"][m],
            "dsb_t": data["dsb"][m],
        }
        for m in range(NCORES)
    ]
    res = run_bass_kernel_spmd(nc, in_maps, list(range(NCORES))).results

    LAST.clear()
    LAST.update(nc=nc, in_maps=in_maps, meta=meta)

    out = np.empty((N, dout), dtype=np.float32)
    perm = data["perm"]
    pos = np.arange(meta["npc"], dtype=np.int64)
    for m in range(NCORES):
        out[perm[pos * NCORES + m]] = res[m]["out_t"][: meta["npc"]]
    return out
